# revision 15
# baseline (speedup 1.0000x reference)
"""Trainium2 Bass kernel for BilateralSlicer (fused trilinear bilateral-grid slice).

out[b,c,y,x] = sum over 2x2x2 taps of grid, with spatial (y,x) bilinear
upsample 16x16 -> 1080x1920 and per-pixel depth lerp driven by guidance.

Strategy (8 cores, full I/O):
  - Shard the 2*1080 = 2160 output rows: core i gets rows [i*270, (i+1)*270).
  - Host precomputes, per output row, the y-lerped table
      T[c,k,j]   (12 x 8 x 16)      and its depth/space diffs
      TD = T[k+1]-T[k],  DjT = T[:, :, j+1]-T[:, :, j],  DjTD likewise,
    interleaved as 4-float chunks  T4[row, cpad16, (k,j), 4].
  - Device, per 8-row block (one row per 16-partition group):
      idx = 16*floor(7*guidance) + jlo(x)  (computed on DVE, fp32-exact)
      E   = gpsimd.ap_gather(T4_block, idx)           # (128, 1920, 4)
      f_rep = PE selection-matmul broadcast of f = frac(7*guidance)
      out = (E0 + wx*E1) + f*(E2 + wx*E3)             # 6 DVE tensor ops
  - Gather/unshard on host.

Self-contained: hardcodes all shapes; only needs numpy + the in-container
concourse (bass) stack.
"""

import os
import sys
import numpy as np

for _p in ("/opt/trn_rl_repo",):
    if _p not in sys.path:
        sys.path.insert(0, _p)

import concourse.bass as bass
import concourse.mybir as mybir
from concourse import bacc
from concourse import tile
from concourse import library_config
from concourse.bass_utils import run_bass_kernel_spmd

F32 = mybir.dt.float32
I16 = mybir.dt.int16

B, C, D, HG, WG = 2, 12, 8, 16, 16
HH, WH = 1080, 1920
NCORES = 8
ROWS_TOTAL = B * HH            # 2160
RPC = ROWS_TOTAL // NCORES     # 270 rows per core
BLK = 8                        # rows per device block (one per gather group)
NBLK = (RPC + BLK - 1) // BLK  # 34 (last block has 6 active rows)
CP = 16                        # padded channels per group
NE = D * WG                    # 128 table entries (k,j)
DCH = 4                        # chunk: [T, DjT, TD, DjTD]
MAGIC = np.float32(12582912.0)  # 1.5 * 2**23

LAST_EXEC_NS = None
LAST_PROFILE = None


def _src_coords(out_size, in_size):
    """fp32 mirror of reference._src_coords (PyTorch bilinear, align_corners=False)."""
    scale = np.float32(in_size / out_size)
    src = (np.arange(out_size, dtype=np.float32) + np.float32(0.5)) * scale - np.float32(0.5)
    src = np.maximum(src, np.float32(0.0))
    i0 = np.minimum(np.floor(src).astype(np.int32), in_size - 1)
    i1 = np.minimum(i0 + 1, in_size - 1)
    w1 = src - i0.astype(np.float32)
    return i0, i1, w1


def _host_prep(grid, guidance):
    grid = np.ascontiguousarray(grid, dtype=np.float32)
    guidance = np.ascontiguousarray(guidance, dtype=np.float32)

    y0, y1, wy = _src_coords(HH, HG)   # (1080,)
    x0, x1, wx = _src_coords(WH, WG)   # (1920,)

    # --- per-row y-lerped tables ------------------------------------------
    # T_all[b, y, c, k, j]
    g0 = grid[:, :, :, y0, :]                      # (B, C, D, HH, WG)
    g1 = grid[:, :, :, y1, :]
    wyb = wy[None, None, None, :, None].astype(np.float32)
    T_all = ((np.float32(1.0) - wyb) * g0 + wyb * g1).astype(np.float32)
    T_all = np.transpose(T_all, (0, 3, 1, 2, 4))   # (B, HH, C, D, WG)

    TD = np.zeros_like(T_all)
    TD[:, :, :, : D - 1, :] = T_all[:, :, :, 1:, :] - T_all[:, :, :, : D - 1, :]
    jn = np.minimum(np.arange(WG) + 1, WG - 1)
    DjT = T_all[:, :, :, :, jn] - T_all
    DjTD = TD[:, :, :, :, jn] - TD

    # T4[b, y, cpad16, k, j, 4]
    T4 = np.zeros((B, HH, CP, D, WG, DCH), dtype=np.float32)
    T4[:, :, :C, :, :, 0] = T_all
    T4[:, :, :C, :, :, 1] = DjT
    T4[:, :, :C, :, :, 2] = TD
    T4[:, :, :C, :, :, 3] = DjTD
    T4 = T4.reshape(ROWS_TOTAL, CP, NE * DCH)      # (2160, 16, 512)

    # --- per-core inputs ---------------------------------------------------
    guid_rows = guidance[:, 0].reshape(ROWS_TOTAL, WH)  # (2160, 1920)

    t4_cores, gw_cores, gf_cores = [], [], []
    for core in range(NCORES):
        r0 = core * RPC
        rows = np.arange(r0, r0 + RPC)

        # T4 per block: (NBLK, 128, 512); partition p = g*16 + cc
        t4c = np.zeros((NBLK, BLK * CP, NE * DCH), dtype=np.float32)
        for t in range(NBLK):
            g_active = min(BLK, RPC - t * BLK)
            blk = T4[r0 + t * BLK : r0 + t * BLK + g_active]      # (g, 16, 512)
            t4c[t, : g_active * CP] = blk.reshape(g_active * CP, NE * DCH)
        t4_cores.append(t4c)

        # wrapped guidance: gw[g*16+q, t*120+s] = guid[row(t,g), s*16+q]
        gwc = np.zeros((128, NBLK * (WH // 16)), dtype=np.float32)
        gr = guid_rows[rows]                                       # (270, 1920)
        grr = gr.reshape(RPC, WH // 16, 16)                        # (270, 120, 16)
        for t in range(NBLK):
            g_active = min(BLK, RPC - t * BLK)
            # (g, 120, 16) -> partitions g*16+q, cols s
            blk = grr[t * BLK : t * BLK + g_active]
            gwc_blk = np.transpose(blk, (0, 2, 1)).reshape(g_active * 16, WH // 16)
            gwc[: g_active * 16, t * (WH // 16) : (t + 1) * (WH // 16)] = gwc_blk
        gw_cores.append(gwc)

        gf_cores.append(np.ascontiguousarray(gr))                  # (270, 1920)

    # --- static tiles (same for every core) --------------------------------
    NW = NBLK * (WH // 16)  # 4080
    jlo_w = np.zeros((128, WH // 16), dtype=np.float32)
    for q in range(16):
        jlo_w[q::16, :] = x0[np.arange(WH // 16) * 16 + q][None, :]
    jlo_big = np.tile(jlo_w, (1, NBLK)).astype(np.float32)         # (128, 4080)

    wx_tile = np.broadcast_to(wx[None, :], (128, WH)).astype(np.float32).copy()

    sel = np.zeros((BLK, 128), dtype=np.float32)
    for g in range(BLK):
        sel[g, g * CP : (g + 1) * CP] = 1.0

    return t4_cores, gw_cores, gf_cores, jlo_big, wx_tile, sel


# ----------------------------------------------------------------------------
# Bass program (SPMD, one program for all 8 cores)
# ----------------------------------------------------------------------------

_NC_CACHE = None


def _build_nc():
    global _NC_CACHE
    if _NC_CACHE is not None:
        return _NC_CACHE

    NW = NBLK * (WH // 16)  # 4080
    nc = bacc.Bacc("TRN2", target_bir_lowering=False, debug=True)

    t4_in = nc.dram_tensor("t4", [NBLK, BLK * CP, NE * DCH], F32, kind="ExternalInput")
    gw_in = nc.dram_tensor("gw", [128, NW], F32, kind="ExternalInput")
    gf_in = nc.dram_tensor("gf", [RPC, WH], F32, kind="ExternalInput")
    jlo_in = nc.dram_tensor("jlo", [128, NW], F32, kind="ExternalInput")
    wx_in = nc.dram_tensor("wx", [128, WH], F32, kind="ExternalInput")
    sel_in = nc.dram_tensor("sel", [BLK, 128], F32, kind="ExternalInput")
    out_d = nc.dram_tensor("out", [NBLK, 128, WH], F32, kind="ExternalOutput")

    ALU = mybir.AluOpType

    with tile.TileContext(nc) as tc:
        nc.gpsimd.load_library(library_config.ap_gather)

        with (
            tc.tile_pool(name="static", bufs=1) as statics,
            tc.tile_pool(name="guid", bufs=1) as guid_pool,
            tc.tile_pool(name="work", bufs=2) as work,
            tc.tile_pool(name="epool", bufs=1) as epool,
            tc.tile_pool(name="inter", bufs=1) as inter,
            tc.tile_pool(name="psum", bufs=2, space="PSUM") as psum,
        ):
            wx_t = statics.tile([128, WH], F32, tag="wx")
            sel_t = statics.tile([BLK, 128], F32, tag="sel")
            jlo_t = statics.tile([128, NW], F32, tag="jlo")
            nc.sync.dma_start(out=wx_t[:], in_=wx_in[:])
            nc.sync.dma_start(out=sel_t[:], in_=sel_in[:])
            nc.sync.dma_start(out=jlo_t[:], in_=jlo_in[:])

            # ---- guidance flat -> f tiles (persist) -------------------------
            f_tiles = []
            row_splits = [(0, 128), (128, 128), (256, RPC - 256)]
            for i, (p0, pn) in enumerate(row_splits):
                gfi = guid_pool.tile([128, WH], F32, tag=f"gf{i}")
                fi = guid_pool.tile([128, WH], F32, tag=f"f{i}")
                tmp = inter.tile([128, WH], F32, tag="gtmp")
                nc.sync.dma_start(out=gfi[:pn], in_=gf_in[p0 : p0 + pn])
                if pn < 128:
                    nc.vector.memset(fi[:], 0.0)
                # t = 7g - 0.5 ; klo = (t + M) - M ; f = 7g - klo
                nc.vector.tensor_scalar(
                    out=tmp[:pn], in0=gfi[:pn], scalar1=7.0, scalar2=0.5,
                    op0=ALU.mult, op1=ALU.subtract)
                nc.vector.tensor_scalar(
                    out=tmp[:pn], in0=tmp[:pn], scalar1=float(MAGIC), scalar2=float(MAGIC),
                    op0=ALU.add, op1=ALU.subtract)
                nc.vector.scalar_tensor_tensor(
                    out=fi[:pn], in0=gfi[:pn], scalar=7.0, in1=tmp[:pn],
                    op0=ALU.mult, op1=ALU.subtract)
                f_tiles.append(fi)

            # ---- wrapped guidance -> int16 gather indices (in-place on gw) --
            gw_t = guid_pool.tile([128, NW], F32, tag="gw")
            idx16 = guid_pool.tile([128, NW], I16, tag="idx16")
            nc.sync.dma_start(out=gw_t[:], in_=gw_in[:])
            nc.vector.tensor_scalar(
                out=gw_t[:], in0=gw_t[:], scalar1=7.0, scalar2=0.5,
                op0=ALU.mult, op1=ALU.subtract)
            nc.vector.tensor_scalar(
                out=gw_t[:], in0=gw_t[:], scalar1=float(MAGIC), scalar2=float(MAGIC),
                op0=ALU.add, op1=ALU.subtract)
            # idx = 16*klo + jlo
            nc.vector.scalar_tensor_tensor(
                out=gw_t[:], in0=gw_t[:], scalar=16.0, in1=jlo_t[:],
                op0=ALU.mult, op1=ALU.add)
            nc.vector.tensor_copy(out=idx16[:], in_=gw_t[:])

            # ---- main block loop -------------------------------------------
            NX = WH // 16  # 120 idx cols per block
            for t in range(NBLK):
                t4_t = work.tile([BLK * CP, NE * DCH], F32, tag="t4")
                e_t = epool.tile([128, WH * DCH], F32, tag="E")
                nc.sync.dma_start(out=t4_t[:], in_=t4_in[t])

                nc.gpsimd.ap_gather(
                    e_t[:], t4_t[:], idx16[:, t * NX : (t + 1) * NX],
                    channels=128, num_elems=NE, d=DCH, num_idxs=WH)

                # f_rep via PE broadcast: (8,128).T @ (8,1920) -> (128,1920)
                ti = (t * BLK) // 128
                po = (t * BLK) % 128
                f_blk = work.tile([BLK, WH], F32, tag="fblk")
                nc.sync.dma_start(out=f_blk[:], in_=f_tiles[ti][po : po + BLK])
                f_rep = psum.tile([128, WH], F32, tag="frep")
                for c0 in range(0, WH, 512):
                    c1 = min(c0 + 512, WH)
                    nc.tensor.matmul(
                        out=f_rep[:, c0:c1], lhsT=sel_t[:], rhs=f_blk[:, c0:c1],
                        start=True, stop=True)

                e0 = e_t[:, 0 : WH * DCH : DCH]
                e1 = e_t[:, 1 : WH * DCH : DCH]
                e2 = e_t[:, 2 : WH * DCH : DCH]
                e3 = e_t[:, 3 : WH * DCH : DCH]

                m1 = inter.tile([128, WH], F32, tag="m1")
                p_t = inter.tile([128, WH], F32, tag="P")
                q_t = inter.tile([128, WH], F32, tag="Q")
                o_t = work.tile([128, WH], F32, tag="O")

                nc.vector.tensor_tensor(out=m1[:], in0=e1, in1=wx_t[:], op=ALU.mult)
                nc.vector.tensor_tensor(out=p_t[:], in0=e0, in1=m1[:], op=ALU.add)
                nc.vector.tensor_tensor(out=m1[:], in0=e3, in1=wx_t[:], op=ALU.mult)
                nc.vector.tensor_tensor(out=q_t[:], in0=e2, in1=m1[:], op=ALU.add)
                nc.vector.tensor_tensor(out=q_t[:], in0=q_t[:], in1=f_rep[:], op=ALU.mult)
                nc.vector.tensor_tensor(out=o_t[:], in0=p_t[:], in1=q_t[:], op=ALU.add)

                # store block (padded layout; host strips channel pads)
                nc.sync.dma_start(out=out_d[t], in_=o_t[:])

    nc.finalize()
    _NC_CACHE = nc
    return nc


def kernel(grid, guidance):
    global LAST_EXEC_NS, LAST_PROFILE
    grid = np.asarray(grid, dtype=np.float32)
    guidance = np.asarray(guidance, dtype=np.float32)

    t4_cores, gw_cores, gf_cores, jlo_big, wx_tile, sel = _host_prep(grid, guidance)

    nc = _build_nc()
    in_maps = []
    for core in range(NCORES):
        in_maps.append({
            "t4": t4_cores[core],
            "gw": gw_cores[core],
            "gf": gf_cores[core],
            "jlo": jlo_big,
            "wx": wx_tile,
            "sel": sel,
        })

    trace = bool(int(os.environ.get("KTRACE", "0")))
    res = run_bass_kernel_spmd(nc, in_maps, core_ids=list(range(NCORES)),
                               trace=trace)
    LAST_EXEC_NS = res.exec_time_ns
    LAST_PROFILE = res.profile_json

    out = np.empty((B, C, HH, WH), dtype=np.float32)
    for core in range(NCORES):
        o = np.asarray(res.results[core]["out"])          # (NBLK, 128, WH)
        o = o.reshape(NBLK, BLK, CP, WH)[:, :, :C, :]     # (NBLK, 8, 12, WH)
        o = o.reshape(NBLK * BLK, C, WH)[:RPC]            # (270, 12, WH)
        r0 = core * RPC
        b = r0 // HH
        y0_ = r0 % HH
        out[b, :, y0_ : y0_ + RPC, :] = np.transpose(o, (1, 0, 2))
    return out
